# revision 7
# baseline (speedup 1.0000x reference)
"""BiGCN (two-branch GCN + global_add_pool + MLP head) on 8 Trainium2 NeuronCores.

Strategy (node-parallel with replicated tables):
  - Nodes are sharded across the 8 cores (6250 real + 22 pad rows -> 6272/core).
  - conv1 dense part (x @ W1, scaled by dinv) is computed node-sharded, then
    AllGather replicates the scaled table h' = dinv * (x @ W1) to every core.
  - conv1 aggregation: each core owns the edges whose OUT endpoint lives in its
    node range.  Edge features h'[in_node] are fetched with dma_gather
    (int16 indices -> table split in two 25088-row halves), and scatter-added
    into the 128-row destination tiles with a one-hot selection matrix built
    on the vector engine (iota compare) feeding PSUM matmul accumulation.
  - conv2 + global_add_pool are folded into a single dense matmul with the
    host-precomputed matrix M = P @ A_hat (pool matrix times normalized
    adjacency, incl. self loops):  pooled = (M @ h1r) @ W2 + counts * b2.
    M columns are node-sharded -> each core computes a partial [512,128]
    pooled sum; one AllReduce combines them.
  - The small MLP head runs replicated on every core; core 0's output is used.
"""

import os
import numpy as np
import ml_dtypes

import concourse.bass as bass
import concourse.bacc as bacc
import concourse.mybir as mybir
import concourse.tile as tile
from concourse.vector_clock import ScopedClock
from concourse.bass_utils import run_bass_kernel_spmd

# ---------------------------------------------------------------- constants
N_NODES = 50000
N_EDGES = 800000
N_GRAPHS = 512
IN_FEATS = 256
HIDDEN = 128
OUT_FEATS = 128

NCORES = 8
NPC_REAL = N_NODES // NCORES          # 6250 real nodes per core
NPC = 6272                            # padded nodes per core (49 * 128)
NTILES = NPC // 128                   # 49
NPAD = NPC * NCORES                   # 50176
SPLIT = 3200                          # rows per core in half A (25 tiles)
HALF_A = NCORES * SPLIT               # 25600
HALF_B = NCORES * (NPC - SPLIT)       # 24576

GCH = 16                              # chunks (of 128 edges) per dma_gather
SEL_B = 16                             # chunks per batched eq op
F32 = mybir.dt.float32
BF16 = mybir.dt.bfloat16
I16 = mybir.dt.int16

_TRACE = os.environ.get("BIGCN_TRACE", "0") == "1"


def _patch_tile_drain():
    """This walrus build rejects a Drain instruction carrying >1 sem wait.
    Split the kernel-tail drain waits across individual sync NOPs."""
    if getattr(tile.TileContext, "_bigcn_drain_patched", False):
        return

    def _drain_and_barrier(self, tick_clock, wait_clock):
        nc = self.nc
        probe = nc.sync.nop(nofuse=True, hint="drain_wait_split")
        wait_clock.add_sem_waits(probe.ins, ScopedClock({None: tick_clock.global_clock}))
        si = probe.ins.sync_info
        waits = list(si.on_wait or []) if si is not None else []
        if len(waits) > 1:
            si.on_wait = waits[:1]
            for w in waits[1:]:
                n2 = nc.sync.nop(nofuse=True, hint="drain_wait_split")
                if n2.ins.sync_info is None:
                    n2.ins.sync_info = mybir.SyncInfo(on_wait=[w], on_update=[])
                else:
                    n2.ins.sync_info.on_wait = [w]
        nc.sync.drain()
        nc.all_engine_barrier()
        assert self.sems is not None
        popped = nc._tile_sem_poison_stack.pop()
        assert popped is self._sem_poison
        nc.clear_and_free_semaphores(list(self.sems.allocated().values()))
        nc.all_engine_barrier()

    tile.TileContext._drain_and_barrier = _drain_and_barrier
    tile.TileContext._bigcn_drain_patched = True


# ---------------------------------------------------------------- host prep
def _pad_id(node):
    """Map a real node id to its padded table row id."""
    return (node // NPC_REAL) * NPC + (node % NPC_REAL)


def _build_edge_streams(out_node, in_node):
    """Group a branch's edges by (core, dst tile, src row-half A/B) and pad each
    (tile, half) group to a uniform (max over cores) chunk count.

    Returns (Tch[49, 2] chunk counts, per-core dict with idx16 / dst_rel
    streams for half 0 and 1)."""
    core = out_node // NPC_REAL
    local = out_node - core * NPC_REAL
    tl = local >> 7
    drel = (local & 127).astype(np.int32)
    cin = in_node // NPC_REAL
    plocal = in_node - cin * NPC_REAL
    half = (plocal >= SPLIT).astype(np.int64)
    idx16 = np.where(half == 0, cin * SPLIT + plocal,
                     cin * (NPC - SPLIT) + (plocal - SPLIT)).astype(np.int32)

    key = (core.astype(np.int64) * NTILES + tl) * 2 + half
    order = np.argsort(key, kind="stable")
    key_s = key[order]
    drel_s = drel[order]
    idx_s = idx16[order]
    counts = np.bincount(key_s, minlength=NCORES * NTILES * 2).reshape(
        NCORES, NTILES, 2
    )
    group_off = np.zeros(NCORES * NTILES * 2 + 1, np.int64)
    np.cumsum(counts.reshape(-1), out=group_off[1:])

    Tch = (np.ceil(counts.max(axis=0) / 128.0)).astype(np.int64)  # [49, 2]
    seg_off = np.zeros((NTILES + 1, 2), np.int64)
    np.cumsum(Tch * 128, axis=0, out=seg_off[1:])

    per_core = []
    for c in range(NCORES):
        streams = {}
        for h in (0, 1):
            L = int(seg_off[NTILES, h])
            idx_pad = np.zeros(L, np.int32)
            drel_pad = np.full(L, -1.0, np.float32)
            for t in range(NTILES):
                g = (c * NTILES + t) * 2 + h
                n = int(counts[c, t, h])
                if n:
                    o = int(seg_off[t, h])
                    s = int(group_off[g])
                    idx_pad[o:o + n] = idx_s[s:s + n]
                    drel_pad[o:o + n] = drel_s[s:s + n]
            streams[h] = (idx_pad, drel_pad)
        per_core.append(streams)
    return Tch, per_core


def _wrap_idx(flat, instr_sizes):
    """int16 index array in dma_gather layout: per instruction, partition p
    column j holds flat[e0 + 16*j + (p % 16)], replicated over the 8
    16-partition groups."""
    out = np.zeros((128, len(flat) // 16), np.int16)
    e0 = 0
    for n in instr_sizes:
        blk = flat[e0:e0 + n].reshape(-1, 16).T.astype(np.int16)  # [16, n/16]
        out[:, e0 // 16:(e0 + n) // 16] = np.tile(blk, (8, 1))
        e0 += n
    return out


def _instr_sizes(n_chunks):
    sizes = []
    left = n_chunks
    while left > 0:
        k = min(GCH, left)
        sizes.append(k * 128)
        left -= k
    return sizes


def _prep(x, edge_index, batch, td_W1, bu_W1, td_b2, bu_b2, pw1, pb1):
    """All host-side graph preprocessing. Returns (schedule, per-core inputs,
    shared inputs)."""
    src = np.asarray(edge_index[0], np.int64)
    dst = np.asarray(edge_index[1], np.int64)
    batch = np.asarray(batch, np.int64)

    deg_td = 1.0 + np.bincount(dst, minlength=N_NODES)
    deg_bu = 1.0 + np.bincount(src, minlength=N_NODES)
    dinv_td = (1.0 / np.sqrt(deg_td)).astype(np.float32)
    dinv_bu = (1.0 / np.sqrt(deg_bu)).astype(np.float32)

    sched = {}
    per_core_edges = {}
    # TD branch: out endpoint = dst, in endpoint = src
    sched["td"], per_core_edges["td"] = _build_edge_streams(dst, src)
    # BU branch: flipped edges -> out endpoint = src, in endpoint = dst
    sched["bu"], per_core_edges["bu"] = _build_edge_streams(src, dst)

    # ---- M matrices (pool @ normalized adjacency incl self loops) ----
    pid_all = _pad_id(np.arange(N_NODES))
    Ms = {}
    for br, (o, i, dv) in {
        "td": (dst, src, dinv_td),
        "bu": (src, dst, dinv_bu),
    }.items():
        w = (dv[o] * dv[i]).astype(np.float64)
        flat = batch[o] * NPAD + pid_all[i]
        M = np.bincount(flat, weights=w, minlength=N_GRAPHS * NPAD)
        diag = batch * NPAD + pid_all
        M += np.bincount(diag, weights=(dv * dv).astype(np.float64),
                         minlength=N_GRAPHS * NPAD)
        Ms[br] = M.reshape(N_GRAPHS, NPAD).astype(np.float32)

    # ---- per-core input maps ----
    xT = np.zeros((IN_FEATS, NPAD), np.float32)
    xTr = np.asarray(x, np.float32).T
    dinv_pad = {"td": np.zeros(NPAD, np.float32), "bu": np.zeros(NPAD, np.float32)}
    for c in range(NCORES):
        xT[:, c * NPC:c * NPC + NPC_REAL] = xTr[:, c * NPC_REAL:(c + 1) * NPC_REAL]
        for br, dv in (("td", dinv_td), ("bu", dinv_bu)):
            dinv_pad[br][c * NPC:c * NPC + NPC_REAL] = dv[
                c * NPC_REAL:(c + 1) * NPC_REAL]

    counts = np.bincount(batch, minlength=N_GRAPHS).astype(np.float32)

    in_maps = []
    for c in range(NCORES):
        m = {
            "xT": np.ascontiguousarray(
                xT[:, c * NPC:(c + 1) * NPC].astype(ml_dtypes.bfloat16)),
            "MT_td": np.ascontiguousarray(
                Ms["td"][:, c * NPC:(c + 1) * NPC].T.astype(ml_dtypes.bfloat16)
                .reshape(NTILES, 128, N_GRAPHS).transpose(1, 0, 2)
                .reshape(128, NTILES * N_GRAPHS)),
            "MT_bu": np.ascontiguousarray(
                Ms["bu"][:, c * NPC:(c + 1) * NPC].T.astype(ml_dtypes.bfloat16)
                .reshape(NTILES, 128, N_GRAPHS).transpose(1, 0, 2)
                .reshape(128, NTILES * N_GRAPHS)),
        }
        for br in ("td", "bu"):
            m[f"dinv_{br}"] = np.ascontiguousarray(
                dinv_pad[br][c * NPC:(c + 1) * NPC].reshape(NTILES, 128).T)
            Tch = sched[br]
            for h in (0, 1):
                idx_pad, drel_pad = per_core_edges[br][c][h]
                nch = len(idx_pad) // 128
                m[f"idx_{br}_{h}"] = _wrap_idx(idx_pad, _instr_sizes(nch))
                m[f"drel_{br}_{h}"] = np.ascontiguousarray(
                    drel_pad.reshape(nch, 128).T.astype(ml_dtypes.bfloat16))
        in_maps.append(m)
    return sched, in_maps, counts


# ---------------------------------------------------------------- device code
def _build(nc, sched, weights):
    """Emit the full bass program (identical for every core; all per-core
    differences live in the input tensors)."""
    td_W1, td_b1, td_W2, td_b2, bu_W1, bu_b1, bu_W2, bu_b2, pw1, pb1, pw2, pb2, counts = weights

    nch = {}       # chunks per (branch, half)
    for br in ("td", "bu"):
        Tch = sched[br]
        for h in (0, 1):
            nch[(br, h)] = int(Tch[:, h].sum())

    # ---------------- dram parameters ----------------
    P = {}
    P["xT"] = nc.declare_dram_parameter("xT", [IN_FEATS, NPC], BF16, isOutput=False)
    for br in ("td", "bu"):
        P[f"dinv_{br}"] = nc.declare_dram_parameter(
            f"dinv_{br}", [128, NTILES], F32, isOutput=False)
        P[f"MT_{br}"] = nc.declare_dram_parameter(
            f"MT_{br}", [128, NTILES * N_GRAPHS], BF16, isOutput=False)
        for h in (0, 1):
            n = nch[(br, h)]
            P[f"idx_{br}_{h}"] = nc.declare_dram_parameter(
                f"idx_{br}_{h}", [128, n * 8], I16, isOutput=False)
            P[f"drel_{br}_{h}"] = nc.declare_dram_parameter(
                f"drel_{br}_{h}", [128, n], BF16, isOutput=False)
    out_ext = nc.declare_dram_parameter("out", [OUT_FEATS, N_GRAPHS], F32, isOutput=True)

    # host-side constant tensors shipped as inputs
    consts_np = {}

    def const_input(name, arr):
        arr = np.ascontiguousarray(arr, np.float32)
        consts_np[name] = arr
        P[name] = nc.declare_dram_parameter(name, list(arr.shape), F32, isOutput=False)
        return P[name]

    consts_np["W1cat"] = np.stack([
        np.asarray(td_W1, np.float32).reshape(2, 128, HIDDEN),
        np.asarray(bu_W1, np.float32).reshape(2, 128, HIDDEN)]).astype(
            ml_dtypes.bfloat16)
    P["W1cat"] = nc.declare_dram_parameter(
        "W1cat", [2, 2, 128, HIDDEN], BF16, isOutput=False)
    const_input("W2cat", np.stack([
        np.asarray(td_W2, np.float32), np.asarray(bu_W2, np.float32)]))  # [2,128,128]
    const_input("b1cat", np.stack([
        np.tile(np.asarray(td_b1, np.float32)[None, :], (128, 1)),
        np.tile(np.asarray(bu_b1, np.float32)[None, :], (128, 1))]))     # [2,128,128]
    const_input("iota", np.tile(np.arange(128, dtype=np.float32)[None, :], (128, 1)))
    const_input("ident", np.eye(128, dtype=np.float32))
    const_input("pw1", np.asarray(pw1, np.float32).reshape(2, 128, 256))
    const_input("pw2", np.asarray(pw2, np.float32).reshape(2, 128, 128))
    b2cat = np.concatenate([np.asarray(bu_b2, np.float32),
                            np.asarray(td_b2, np.float32)])
    q1 = b2cat @ np.asarray(pw1, np.float32)  # [256]
    # rank-2 bias rows: m1 += counts (x) q1 + ones (x) pb1
    const_input("q1row", np.stack([q1, np.asarray(pb1, np.float32)]))  # [2, 256]
    const_input("crow", np.stack([np.asarray(counts, np.float32),
                                  np.ones(N_GRAPHS, np.float32)]))  # [2, 512]
    const_input("ones1", np.ones((1, N_GRAPHS), np.float32))
    const_input("pb2row", np.asarray(pb2, np.float32).reshape(1, 128))

    b1_nonzero = {
        "td": bool(np.any(np.asarray(td_b1) != 0)),
        "bu": bool(np.any(np.asarray(bu_b1) != 0)),
    }

    gq = [0]

    def next_q():
        q = gq[0] % 4
        gq[0] += 1
        return q

    with tile.TileContext(nc) as tc:
        with tc.tile_pool(name="dram", bufs=1, space="DRAM") as dram, \
             tc.tile_pool(name="const", bufs=1) as constp, \
             tc.tile_pool(name="persist", bufs=1) as persist:

            # --------- constants to SBUF ---------
            cw1 = constp.tile([128, 2, 2, 128], BF16, name="cw1")
            nc.sync.dma_start(out=cw1[:], in_=P["W1cat"][:].rearrange(
                "b k p f -> p b k f"))
            cw2 = constp.tile([128, 2, 128], F32, name="cw2")
            nc.sync.dma_start(out=cw2[:], in_=P["W2cat"][:].rearrange("b p f -> p b f"))
            cb1 = constp.tile([128, 2, 128], F32, name="cb1")
            nc.sync.dma_start(out=cb1[:], in_=P["b1cat"][:].rearrange("b p f -> p b f"))
            ciota32 = constp.tile([128, 128], F32, name="ciota32")
            nc.sync.dma_start(out=ciota32[:], in_=P["iota"][:])
            ciota = constp.tile([128, 128], BF16, name="ciota")
            nc.vector.tensor_copy(ciota[:], ciota32[:])
            cident = constp.tile([128, 128], F32, name="cident")
            nc.sync.dma_start(out=cident[:], in_=P["ident"][:])
            cidentb = constp.tile([128, 128], BF16, name="cidentb")
            nc.vector.tensor_copy(cidentb[:], cident[:])
            cpw1 = constp.tile([128, 2, 256], F32, name="cpw1")
            nc.sync.dma_start(out=cpw1[:], in_=P["pw1"][:].rearrange("k p j -> p k j"))
            cpw2 = constp.tile([128, 2, 128], F32, name="cpw2")
            nc.sync.dma_start(out=cpw2[:], in_=P["pw2"][:].rearrange("k p f -> p k f"))
            cq1 = constp.tile([2, 256], F32, name="cq1")
            nc.sync.dma_start(out=cq1[:], in_=P["q1row"][:])
            ccrow = constp.tile([2, N_GRAPHS], F32, name="ccrow")
            nc.sync.dma_start(out=ccrow[:], in_=P["crow"][:])
            cones = constp.tile([1, N_GRAPHS], F32, name="cones")
            nc.sync.dma_start(out=cones[:], in_=P["ones1"][:])
            cpb2 = constp.tile([1, 128], F32, name="cpb2")
            nc.sync.dma_start(out=cpb2[:], in_=P["pb2row"][:])
            cdinv = {}
            for br in ("td", "bu"):
                cdinv[br] = constp.tile([128, NTILES], F32, name=f"cdinv{br}")
                nc.sync.dma_start(out=cdinv[br][:], in_=P[f"dinv_{br}"][:])

            # --------- dram intermediates ---------
            agin2 = dram.tile([NPC, 2 * HIDDEN], BF16, name="agin2")
            hg2sA = dram.tile([HALF_A, 2 * HIDDEN], BF16, name="hg2sA",
                              addr_space="Shared")
            hg2sB = dram.tile([HALF_B, 2 * HIDDEN], BF16, name="hg2sB",
                              addr_space="Shared")
            ar_in = {}
            ar_out = {}
            for br in ("td", "bu"):
                ar_in[br] = dram.tile([128, N_GRAPHS], BF16, name=f"ar_in{br}")
                ar_out[br] = dram.tile([128, N_GRAPHS], BF16, name=f"ar_out{br}",
                                       addr_space="Shared")

            hploc = persist.tile([128, NTILES, 2 * HIDDEN], BF16, name="hploc")

            # =========== phase A: dense h' = dinv * (x @ W1), both branches ===========
            with tc.tile_pool(name="xT", bufs=1) as xp, \
                 tc.tile_pool(name="psA", bufs=2, space="PSUM") as psA:
                xt = xp.tile([128, 2, NPC], BF16, name="xt")
                for q in range(4):
                    nc.sync.dma_start(
                        out=xt[:, :, q * (NPC // 4):(q + 1) * (NPC // 4)],
                        in_=P["xT"][:].rearrange("(k p) n -> p k n", p=128)[
                            :, :, q * (NPC // 4):(q + 1) * (NPC // 4)])
                for t in range(NTILES):
                    for bi, br in enumerate(("td", "bu")):
                        ps = psA.tile([128, 128], F32, space="PSUM", tag="psA")
                        for k in range(2):
                            nc.tensor.matmul(
                                out=ps[:],
                                lhsT=xt[:, k, t * 128:(t + 1) * 128],
                                rhs=cw1[:, bi, k, :],
                                start=(k == 0), stop=(k == 1),
                            )
                        nc.scalar.activation(
                            out=hploc[:, t, bi * HIDDEN:(bi + 1) * HIDDEN],
                            in_=ps[:],
                            func=mybir.ActivationFunctionType.Copy,
                            scale=cdinv[br][:, t:t + 1])
                        nc.sync.dma_start(
                            out=agin2[t * 128:(t + 1) * 128,
                                      bi * HIDDEN:(bi + 1) * HIDDEN],
                            in_=hploc[:, t, bi * HIDDEN:(bi + 1) * HIDDEN])
                # Two AllGathers: half A (each core's first 25 tiles) fires as
                # soon as those agin2 rows are written; half B follows.  The
                # Shared->Local copy of A overlaps the half-B AllGather, and
                # half-A gathers overlap the half-B copy.
                nc.gpsimd.collective_compute(
                    "AllGather", mybir.AluOpType.bypass,
                    replica_groups=[list(range(NCORES))],
                    ins=[agin2[0:SPLIT, :].opt()],
                    outs=[hg2sA[:].opt()],
                )
                nc.gpsimd.collective_compute(
                    "AllGather", mybir.AluOpType.bypass,
                    replica_groups=[list(range(NCORES))],
                    ins=[agin2[SPLIT:NPC, :].opt()],
                    outs=[hg2sB[:].opt()],
                )
                hg2_halves = []
                for hh, (shared, hrows) in enumerate(
                        ((hg2sA, HALF_A), (hg2sB, HALF_B))):
                    hloc = dram.tile([hrows, 2 * HIDDEN], BF16, name=f"hg2l{hh}")
                    for rr in range(0, hrows, hrows // 2):
                        nc.sync.dma_start(
                            out=hloc[rr:rr + hrows // 2, :],
                            in_=shared[rr:rr + hrows // 2, :])
                    hg2_halves.append(hloc)

            # =========== phase B/C: per-branch aggregation + conv2/pool ===========
            with tc.tile_pool(name="psG", bufs=2, space="PSUM") as psG, \
                 tc.tile_pool(name="psY", bufs=1, space="PSUM") as psY, \
                 tc.tile_pool(name="psT", bufs=1, space="PSUM") as psT, \
                 tc.tile_pool(name="idxp", bufs=2) as idxp, \
                 tc.tile_pool(name="stag", bufs=12) as stag, \
                 tc.tile_pool(name="selp", bufs=4) as selp, \
                 tc.tile_pool(name="accp", bufs=1) as accp, \
                 tc.tile_pool(name="h1rp", bufs=1) as h1rp, \
                 tc.tile_pool(name="mtp", bufs=3) as mtp, \
                 tc.tile_pool(name="misc", bufs=2) as misc:

                acc = {}
                h1r = {}
                pooledT_sb = {}
                seg = {}
                for br in ("td", "bu"):
                    acc[br] = accp.tile([128, NTILES, 128], BF16, name=f"acc{br}")
                    h1r[br] = h1rp.tile([128, NTILES, 128], BF16, name=f"h1r{br}")
                    Tch = sched[br]
                    so = np.zeros((NTILES + 1, 2), np.int64)
                    np.cumsum(Tch * 128, axis=0, out=so[1:])
                    seg[br] = so

                idx_max = max(nch[(b2_, h2_)] for b2_ in ("td", "bu")
                              for h2_ in (0, 1))
                emit_ar_td = [None]
                psy = None
                # pass order A-halves first (their table copy lands first),
                # so descriptor generation never waits on the half-B copy.
                for br, h in (("td", 0), ("bu", 0), ("td", 1), ("bu", 1)):
                    bi = 0 if br == "td" else 1
                    seg_off = seg[br]
                    if h == 1 and br == "td":
                        psy = [psY.tile([128, 128], F32, space="PSUM",
                                        tag=f"psY{g}", name=f"psytd{g}")
                               for g in range(4)]
                    elif h == 1 and br == "bu":
                        psy = [psY.tile([128, 128], F32, space="PSUM",
                                        tag=f"psY{g}", name=f"psybu{g}")
                               for g in range(4)]
                    if True:
                        n = nch[(br, h)]
                        idx_sb_h = idxp.tile([128, idx_max * 8], I16,
                                             tag="idx", name=f"idx{br}{h}")
                        nc.sync.dma_start(out=idx_sb_h[:, :n * 8],
                                          in_=P[f"idx_{br}_{h}"][:])
                        drel_sb_h = idxp.tile([128, idx_max], BF16,
                                              tag="drel", name=f"drel{br}{h}")
                        nc.sync.dma_start(out=drel_sb_h[:, :n],
                                          in_=P[f"drel_{br}_{h}"][:])
                        idx_sb = {h: idx_sb_h}
                        drel_sb = {h: drel_sb_h}
                        n_chunks = nch[(br, h)]
                        sizes = _instr_sizes(n_chunks)
                        bi_ = 0 if br == "td" else 1
                        table = hg2_halves[h][:, bi_ * HIDDEN:(bi_ + 1) * HIDDEN]

                        # gather instructions
                        stage_tiles = []
                        e0 = 0
                        gi = 0
                        for n in sizes:
                            st = stag.tile([128, GCH * 128], BF16, tag="stag")
                            nc.gpsimd.dma_gather(
                                out_ap=st[:, :n].rearrange(
                                    "p (c e) -> p c e", e=128),
                                in_ap=table,
                                idxs_ap=idx_sb[h][:, e0 // 16:(e0 + n) // 16],
                                num_idxs=n, num_idxs_reg=n, elem_size=128,
                                elem_step=2 * HIDDEN,
                                single_packet=False, queue_num=next_q(),
                            )
                            stage_tiles.append((st, e0 // 128, n // 128))
                            e0 += n
                            gi += 1
                            if gi == 10 and emit_ar_td[0] is not None:
                                # fire the td AllReduce from deep inside the
                                # bu half-B gather stream: pooled-td is ready
                                # by then, so the trigger never stalls gpsimd.
                                emit_ar_td[0]()
                                emit_ar_td[0] = None

                        def chunk_slice(c):
                            for st, c0, cn in stage_tiles:
                                if c0 <= c < c0 + cn:
                                    return st[:, (c - c0) * 128:(c - c0 + 1) * 128]
                            raise AssertionError

                        # batched selection-matrix build
                        sel_tiles = {}
                        for c0 in range(0, n_chunks, SEL_B):
                            b = min(SEL_B, n_chunks - c0)
                            sel = selp.tile([128, SEL_B * 128], BF16, tag="sel")
                            nc.vector.tensor_tensor(
                                out=sel[:, :b * 128].rearrange(
                                    "p (c d) -> p c d", d=128),
                                in0=drel_sb[h][:, c0:c0 + b].unsqueeze(2)
                                    .to_broadcast([128, b, 128]),
                                in1=ciota[:].unsqueeze(1).to_broadcast([128, b, 128]),
                                op=mybir.AluOpType.is_equal,
                            )
                            sel_tiles[c0] = sel

                        def sel_slice(c):
                            c0 = (c // SEL_B) * SEL_B
                            j = c - c0
                            return sel_tiles[c0][:, j * 128:(j + 1) * 128]

                        # per-tile PSUM accumulation + eviction (all adds on
                        # PE via identity matmuls; evictions on ACT -- keeps
                        # DVE off the shared SBUF port pair so SWDGE
                        # descriptor generation isn't blocked)
                        for t in range(NTILES):
                            ca, cb_ = int(seg_off[t, h]) // 128, int(seg_off[t + 1, h]) // 128
                            ps = psG.tile([128, 128], F32, space="PSUM", tag="psG")
                            for c in range(ca, cb_):
                                nc.tensor.matmul(
                                    out=ps[:], lhsT=sel_slice(c),
                                    rhs=chunk_slice(c),
                                    start=(c == ca), stop=False,
                                )
                            if h == 0:
                                # psum += h'local[t]; acc[t] = psum (bf16)
                                nc.tensor.matmul(
                                    out=ps[:], lhsT=cidentb[:],
                                    rhs=hploc[:, t, bi * HIDDEN:(bi + 1) * HIDDEN],
                                    start=(ca == cb_), stop=True)
                                nc.scalar.activation(
                                    out=acc[br][:, t, :], in_=ps[:],
                                    func=mybir.ActivationFunctionType.Copy)
                            else:
                                # psum += acc[t]; h1r[t] = relu(dinv * psum)
                                nc.tensor.matmul(
                                    out=ps[:], lhsT=cidentb[:], rhs=acc[br][:, t, :],
                                    start=(ca == cb_), stop=True)
                                if b1_nonzero[br]:
                                    tmp2 = misc.tile([128, 128], F32, tag="tmp2")
                                    nc.scalar.activation(
                                        out=tmp2[:], in_=ps[:],
                                        func=mybir.ActivationFunctionType.Copy,
                                        scale=cdinv[br][:, t:t + 1])
                                    nc.vector.tensor_add(tmp2[:], tmp2[:], cb1[:, bi, :])
                                    nc.scalar.activation(
                                        out=h1r[br][:, t, :], in_=tmp2[:],
                                        func=mybir.ActivationFunctionType.Relu)
                                else:
                                    nc.scalar.activation(
                                        out=h1r[br][:, t, :], in_=ps[:],
                                        func=mybir.ActivationFunctionType.Relu,
                                        scale=cdinv[br][:, t:t + 1])
                                # conv2+pool partial: Y[g] += MT[t].T slices @ h1r[t]
                                if t % 2 == 0:
                                    tn = min(2, NTILES - t)
                                    mt = mtp.tile([128, 2 * N_GRAPHS], BF16, tag="mt")
                                    nc.sync.dma_start(
                                        out=mt[:, :tn * N_GRAPHS],
                                        in_=P[f"MT_{br}"][
                                            :, t * N_GRAPHS:(t + tn) * N_GRAPHS])
                                mtoff = (t % 2) * N_GRAPHS
                                for g in range(4):
                                    nc.tensor.matmul(
                                        out=psy[g][:],
                                        lhsT=mt[:, mtoff + g * 128:mtoff + (g + 1) * 128],
                                        rhs=h1r[br][:, t, :],
                                        start=(t == 0), stop=(t == NTILES - 1),
                                        skip_group_check=True,
                                    )

                    if h != 1:
                        continue
                    # transpose Y -> YT [128f, 512g]
                    yt = misc.tile([128, N_GRAPHS], F32, tag="yt")
                    for g in range(4):
                        ysb = misc.tile([128, 128], F32, tag="ysb")
                        nc.scalar.activation(out=ysb[:], in_=psy[g][:],
                                             func=mybir.ActivationFunctionType.Copy)
                        pst = psT.tile([128, 128], F32, space="PSUM", tag="psT")
                        nc.tensor.transpose(out=pst[:], in_=ysb[:], identity=cident[:])
                        nc.scalar.activation(out=yt[:, g * 128:(g + 1) * 128],
                                             in_=pst[:],
                                             func=mybir.ActivationFunctionType.Copy)
                    # pooledT = W2^T-contraction: [128fo, 512g]
                    psp = psT.tile([128, N_GRAPHS], F32, space="PSUM", tag="psp")
                    nc.tensor.matmul(out=psp[:], lhsT=cw2[:, bi, :], rhs=yt[:],
                                     start=True, stop=True)
                    pooledT_sb[br] = misc.tile([128, N_GRAPHS], BF16, tag=f"pool{br}", name=f"pool{br}")
                    nc.scalar.activation(out=pooledT_sb[br][:], in_=psp[:],
                                         func=mybir.ActivationFunctionType.Copy)
                    nc.sync.dma_start(out=ar_in[br][:], in_=pooledT_sb[br][:])

                    def _mk_ar(br=br):
                        def _emit():
                            nc.gpsimd.collective_compute(
                                "AllReduce", mybir.AluOpType.add,
                                replica_groups=[list(range(NCORES))],
                                ins=[ar_in[br][:].opt()],
                                outs=[ar_out[br][:].opt()],
                            )
                        return _emit
                    if br == "td":
                        emit_ar_td[0] = _mk_ar()
                    else:
                        _mk_ar()()

            # =========== phase D: MLP head (replicated) ===========
            with tc.tile_pool(name="psM", bufs=1, space="PSUM") as psM, \
                 tc.tile_pool(name="mlp", bufs=1) as mlp:
                catb = mlp.tile([128, 2, N_GRAPHS], BF16, name="catb")
                # cat order is [bu, td] -> slot 0 = bu, slot 1 = td
                nc.sync.dma_start(out=catb[:, 0, :], in_=ar_out["bu"][:])
                nc.sync.dma_start(out=catb[:, 1, :], in_=ar_out["td"][:])
                cat = mlp.tile([128, 2, N_GRAPHS], F32, name="cat")
                nc.vector.tensor_copy(cat[:], catb[:])
                m1 = []
                for j in range(2):
                    pm = psM.tile([128, N_GRAPHS], F32, space="PSUM", tag=f"psM{j}", name=f"pm{j}")
                    for k in range(2):
                        nc.tensor.matmul(
                            out=pm[:], lhsT=cpw1[:, k, j * 128:(j + 1) * 128],
                            rhs=cat[:, k, :], start=(k == 0), stop=False,
                            skip_group_check=True)
                    # rank-2 bias: [q1; pb1-via-q1? q1 already includes pb1] x [counts; ones]
                    nc.tensor.matmul(
                        out=pm[:], lhsT=cq1[:2, j * 128:(j + 1) * 128],
                        rhs=ccrow[:2, :], start=False, stop=True,
                        skip_group_check=True)
                    m1t = mlp.tile([128, N_GRAPHS], F32, name=f"m1t{j}")
                    nc.scalar.activation(out=m1t[:], in_=pm[:],
                                         func=mybir.ActivationFunctionType.Relu)
                    m1.append(m1t)
                pm2 = psM.tile([128, N_GRAPHS], F32, space="PSUM", tag="psM2")
                for j in range(2):
                    nc.tensor.matmul(out=pm2[:], lhsT=cpw2[:, j, :], rhs=m1[j][:],
                                     start=(j == 0), stop=False,
                                     skip_group_check=True)
                nc.tensor.matmul(out=pm2[:], lhsT=cpb2[:1, :], rhs=cones[:1, :],
                                 start=False, stop=True, skip_group_check=True)
                o_sb = mlp.tile([128, N_GRAPHS], F32, name="o_sb")
                nc.vector.tensor_copy(o_sb[:], pm2[:])
                nc.sync.dma_start(out=out_ext[:], in_=o_sb[:])

    return consts_np


# ---------------------------------------------------------------- entrypoint
def kernel(x, edge_index, batch, num_graphs,
           td_W1, td_b1, td_W2, td_b2,
           bu_W1, bu_b1, bu_W2, bu_b2,
           pw1, pb1, pw2, pb2):
    _patch_tile_drain()
    x = np.asarray(x)
    edge_index = np.asarray(edge_index)
    batch = np.asarray(batch)

    counts = np.bincount(np.asarray(batch, np.int64),
                         minlength=N_GRAPHS).astype(np.float32)
    sched, in_maps, counts = _prep(x, edge_index, batch, td_W1, bu_W1,
                                   td_b2, bu_b2, pw1, pb1)

    nc = bacc.Bacc("TRN2", num_devices=NCORES, num_swdge_queues=4)
    weights = (td_W1, td_b1, td_W2, td_b2, bu_W1, bu_b1, bu_W2, bu_b2,
               pw1, pb1, pw2, pb2, counts)
    consts_np = _build(nc, sched, weights)
    nc.finalize()

    for m in in_maps:
        m.update(consts_np)

    core_ids = list(range(NCORES))
    kw = {}
    td = os.environ.get("BIGCN_TMPDIR")
    if td:
        os.makedirs(td, exist_ok=True)
        kw["tmpdir"] = td
    res = run_bass_kernel_spmd(nc, in_maps, core_ids, trace=_TRACE, **kw)
    if _TRACE and res.exec_time_ns is not None:
        print(f"HW exec time: {res.exec_time_ns} ns")

    outT = res.results[0]["out"]          # [128 feat, 512 graphs]
    return np.ascontiguousarray(outT.T).astype(np.float32)



# revision 8
# speedup vs baseline: 1.0687x; 1.0687x over previous
"""BiGCN (two-branch GCN + global_add_pool + MLP head) on 8 Trainium2 NeuronCores.

Strategy (node-parallel with replicated tables):
  - Nodes are sharded across the 8 cores (6250 real + 22 pad rows -> 6272/core).
  - conv1 dense part (x @ W1, scaled by dinv) is computed node-sharded, then
    AllGather replicates the scaled table h' = dinv * (x @ W1) to every core.
  - conv1 aggregation: each core owns the edges whose OUT endpoint lives in its
    node range.  Edge features h'[in_node] are fetched with dma_gather
    (int16 indices -> table split in two 25088-row halves), and scatter-added
    into the 128-row destination tiles with a one-hot selection matrix built
    on the vector engine (iota compare) feeding PSUM matmul accumulation.
  - conv2 + global_add_pool are folded into a single dense matmul with the
    host-precomputed matrix M = P @ A_hat (pool matrix times normalized
    adjacency, incl. self loops):  pooled = (M @ h1r) @ W2 + counts * b2.
    M columns are node-sharded -> each core computes a partial [512,128]
    pooled sum; one AllReduce combines them.
  - The small MLP head runs replicated on every core; core 0's output is used.
"""

import os
import numpy as np
import ml_dtypes

import concourse.bass as bass
import concourse.bacc as bacc
import concourse.mybir as mybir
import concourse.tile as tile
from concourse.vector_clock import ScopedClock
from concourse.bass_utils import run_bass_kernel_spmd

# ---------------------------------------------------------------- constants
N_NODES = 50000
N_EDGES = 800000
N_GRAPHS = 512
IN_FEATS = 256
HIDDEN = 128
OUT_FEATS = 128

NCORES = 8
NPC_REAL = N_NODES // NCORES          # 6250 real nodes per core
NPC = 6272                            # padded nodes per core (49 * 128)
NTILES = NPC // 128                   # 49
NPAD = NPC * NCORES                   # 50176
SPLIT = 3200                          # rows per core in half A (25 tiles)
HALF_A = NCORES * SPLIT               # 25600
HALF_B = NCORES * (NPC - SPLIT)       # 24576

GCH = 16                              # chunks (of 128 edges) per dma_gather
SEL_B = 16                             # chunks per batched eq op
F32 = mybir.dt.float32
BF16 = mybir.dt.bfloat16
I16 = mybir.dt.int16

_TRACE = os.environ.get("BIGCN_TRACE", "0") == "1"


def _patch_tile_drain():
    """This walrus build rejects a Drain instruction carrying >1 sem wait.
    Split the kernel-tail drain waits across individual sync NOPs."""
    if getattr(tile.TileContext, "_bigcn_drain_patched", False):
        return

    def _drain_and_barrier(self, tick_clock, wait_clock):
        nc = self.nc
        probe = nc.sync.nop(nofuse=True, hint="drain_wait_split")
        wait_clock.add_sem_waits(probe.ins, ScopedClock({None: tick_clock.global_clock}))
        si = probe.ins.sync_info
        waits = list(si.on_wait or []) if si is not None else []
        if len(waits) > 1:
            si.on_wait = waits[:1]
            for w in waits[1:]:
                n2 = nc.sync.nop(nofuse=True, hint="drain_wait_split")
                if n2.ins.sync_info is None:
                    n2.ins.sync_info = mybir.SyncInfo(on_wait=[w], on_update=[])
                else:
                    n2.ins.sync_info.on_wait = [w]
        nc.sync.drain()
        nc.all_engine_barrier()
        assert self.sems is not None
        popped = nc._tile_sem_poison_stack.pop()
        assert popped is self._sem_poison
        nc.clear_and_free_semaphores(list(self.sems.allocated().values()))
        nc.all_engine_barrier()

    tile.TileContext._drain_and_barrier = _drain_and_barrier
    tile.TileContext._bigcn_drain_patched = True


# ---------------------------------------------------------------- host prep
def _pad_id(node):
    """Map a real node id to its padded table row id."""
    return (node // NPC_REAL) * NPC + (node % NPC_REAL)


def _build_edge_streams(out_node, in_node):
    """Group a branch's edges by (core, dst tile, src row-half A/B) and pad each
    (tile, half) group to a uniform (max over cores) chunk count.

    Returns (Tch[49, 2] chunk counts, per-core dict with idx16 / dst_rel
    streams for half 0 and 1)."""
    core = out_node // NPC_REAL
    local = out_node - core * NPC_REAL
    tl = local >> 7
    drel = (local & 127).astype(np.int32)
    cin = in_node // NPC_REAL
    plocal = in_node - cin * NPC_REAL
    half = (plocal >= SPLIT).astype(np.int64)
    idx16 = np.where(half == 0, cin * SPLIT + plocal,
                     cin * (NPC - SPLIT) + (plocal - SPLIT)).astype(np.int32)

    key = (core.astype(np.int64) * NTILES + tl) * 2 + half
    order = np.argsort(key, kind="stable")
    key_s = key[order]
    drel_s = drel[order]
    idx_s = idx16[order]
    counts = np.bincount(key_s, minlength=NCORES * NTILES * 2).reshape(
        NCORES, NTILES, 2
    )
    group_off = np.zeros(NCORES * NTILES * 2 + 1, np.int64)
    np.cumsum(counts.reshape(-1), out=group_off[1:])

    Tch = (np.ceil(counts.max(axis=0) / 128.0)).astype(np.int64)  # [49, 2]
    seg_off = np.zeros((NTILES + 1, 2), np.int64)
    np.cumsum(Tch * 128, axis=0, out=seg_off[1:])

    per_core = []
    for c in range(NCORES):
        streams = {}
        for h in (0, 1):
            L = int(seg_off[NTILES, h])
            idx_pad = np.zeros(L, np.int32)
            drel_pad = np.full(L, -1.0, np.float32)
            for t in range(NTILES):
                g = (c * NTILES + t) * 2 + h
                n = int(counts[c, t, h])
                if n:
                    o = int(seg_off[t, h])
                    s = int(group_off[g])
                    idx_pad[o:o + n] = idx_s[s:s + n]
                    drel_pad[o:o + n] = drel_s[s:s + n]
            streams[h] = (idx_pad, drel_pad)
        per_core.append(streams)
    return Tch, per_core


def _wrap_idx(flat, instr_sizes):
    """int16 index array in dma_gather layout: per instruction, partition p
    column j holds flat[e0 + 16*j + (p % 16)], replicated over the 8
    16-partition groups."""
    out = np.zeros((128, len(flat) // 16), np.int16)
    e0 = 0
    for n in instr_sizes:
        blk = flat[e0:e0 + n].reshape(-1, 16).T.astype(np.int16)  # [16, n/16]
        out[:, e0 // 16:(e0 + n) // 16] = np.tile(blk, (8, 1))
        e0 += n
    return out


def _instr_sizes(n_chunks):
    sizes = []
    left = n_chunks
    while left > 0:
        k = min(GCH, left)
        sizes.append(k * 128)
        left -= k
    return sizes


def _prep(x, edge_index, batch, td_W1, bu_W1, td_b2, bu_b2, pw1, pb1):
    """All host-side graph preprocessing. Returns (schedule, per-core inputs,
    shared inputs)."""
    src = np.asarray(edge_index[0], np.int64)
    dst = np.asarray(edge_index[1], np.int64)
    batch = np.asarray(batch, np.int64)

    deg_td = 1.0 + np.bincount(dst, minlength=N_NODES)
    deg_bu = 1.0 + np.bincount(src, minlength=N_NODES)
    dinv_td = (1.0 / np.sqrt(deg_td)).astype(np.float32)
    dinv_bu = (1.0 / np.sqrt(deg_bu)).astype(np.float32)

    sched = {}
    per_core_edges = {}
    # TD branch: out endpoint = dst, in endpoint = src
    sched["td"], per_core_edges["td"] = _build_edge_streams(dst, src)
    # BU branch: flipped edges -> out endpoint = src, in endpoint = dst
    sched["bu"], per_core_edges["bu"] = _build_edge_streams(src, dst)

    # ---- M matrices (pool @ normalized adjacency incl self loops) ----
    pid_all = _pad_id(np.arange(N_NODES))
    Ms = {}
    for br, (o, i, dv) in {
        "td": (dst, src, dinv_td),
        "bu": (src, dst, dinv_bu),
    }.items():
        w = (dv[o] * dv[i]).astype(np.float64)
        flat = batch[o] * NPAD + pid_all[i]
        M = np.bincount(flat, weights=w, minlength=N_GRAPHS * NPAD)
        diag = batch * NPAD + pid_all
        M += np.bincount(diag, weights=(dv * dv).astype(np.float64),
                         minlength=N_GRAPHS * NPAD)
        Ms[br] = M.reshape(N_GRAPHS, NPAD).astype(np.float32)

    # ---- per-core input maps ----
    xT = np.zeros((IN_FEATS, NPAD), np.float32)
    xTr = np.asarray(x, np.float32).T
    dinv_pad = {"td": np.zeros(NPAD, np.float32), "bu": np.zeros(NPAD, np.float32)}
    for c in range(NCORES):
        xT[:, c * NPC:c * NPC + NPC_REAL] = xTr[:, c * NPC_REAL:(c + 1) * NPC_REAL]
        for br, dv in (("td", dinv_td), ("bu", dinv_bu)):
            dinv_pad[br][c * NPC:c * NPC + NPC_REAL] = dv[
                c * NPC_REAL:(c + 1) * NPC_REAL]

    counts = np.bincount(batch, minlength=N_GRAPHS).astype(np.float32)

    in_maps = []
    for c in range(NCORES):
        m = {
            "xT": np.ascontiguousarray(
                xT[:, c * NPC:(c + 1) * NPC].astype(ml_dtypes.bfloat16)),
            "MT_td": np.ascontiguousarray(
                Ms["td"][:, c * NPC:(c + 1) * NPC].T.astype(ml_dtypes.bfloat16)
                .reshape(NTILES, 128, N_GRAPHS).transpose(1, 0, 2)
                .reshape(128, NTILES * N_GRAPHS)),
            "MT_bu": np.ascontiguousarray(
                Ms["bu"][:, c * NPC:(c + 1) * NPC].T.astype(ml_dtypes.bfloat16)
                .reshape(NTILES, 128, N_GRAPHS).transpose(1, 0, 2)
                .reshape(128, NTILES * N_GRAPHS)),
        }
        for br in ("td", "bu"):
            m[f"dinv_{br}"] = np.ascontiguousarray(
                dinv_pad[br][c * NPC:(c + 1) * NPC].reshape(NTILES, 128).T)
            Tch = sched[br]
            for h in (0, 1):
                idx_pad, drel_pad = per_core_edges[br][c][h]
                nch = len(idx_pad) // 128
                m[f"idx_{br}_{h}"] = _wrap_idx(idx_pad, _instr_sizes(nch))
                m[f"drel_{br}_{h}"] = np.ascontiguousarray(
                    drel_pad.reshape(nch, 128).T.astype(ml_dtypes.bfloat16))
        in_maps.append(m)
    return sched, in_maps, counts


# ---------------------------------------------------------------- device code
def _build(nc, sched, weights):
    """Emit the full bass program (identical for every core; all per-core
    differences live in the input tensors)."""
    td_W1, td_b1, td_W2, td_b2, bu_W1, bu_b1, bu_W2, bu_b2, pw1, pb1, pw2, pb2, counts = weights

    nch = {}       # chunks per (branch, half)
    for br in ("td", "bu"):
        Tch = sched[br]
        for h in (0, 1):
            nch[(br, h)] = int(Tch[:, h].sum())

    # ---------------- dram parameters ----------------
    P = {}
    P["xT"] = nc.declare_dram_parameter("xT", [IN_FEATS, NPC], BF16, isOutput=False)
    for br in ("td", "bu"):
        P[f"dinv_{br}"] = nc.declare_dram_parameter(
            f"dinv_{br}", [128, NTILES], F32, isOutput=False)
        P[f"MT_{br}"] = nc.declare_dram_parameter(
            f"MT_{br}", [128, NTILES * N_GRAPHS], BF16, isOutput=False)
        for h in (0, 1):
            n = nch[(br, h)]
            P[f"idx_{br}_{h}"] = nc.declare_dram_parameter(
                f"idx_{br}_{h}", [128, n * 8], I16, isOutput=False)
            P[f"drel_{br}_{h}"] = nc.declare_dram_parameter(
                f"drel_{br}_{h}", [128, n], BF16, isOutput=False)
    out_ext = nc.declare_dram_parameter("out", [OUT_FEATS, N_GRAPHS], F32, isOutput=True)

    # host-side constant tensors shipped as inputs
    consts_np = {}

    def const_input(name, arr):
        arr = np.ascontiguousarray(arr, np.float32)
        consts_np[name] = arr
        P[name] = nc.declare_dram_parameter(name, list(arr.shape), F32, isOutput=False)
        return P[name]

    consts_np["W1cat"] = np.stack([
        np.asarray(td_W1, np.float32).reshape(2, 128, HIDDEN),
        np.asarray(bu_W1, np.float32).reshape(2, 128, HIDDEN)]).astype(
            ml_dtypes.bfloat16)
    P["W1cat"] = nc.declare_dram_parameter(
        "W1cat", [2, 2, 128, HIDDEN], BF16, isOutput=False)
    const_input("W2cat", np.stack([
        np.asarray(td_W2, np.float32), np.asarray(bu_W2, np.float32)]))  # [2,128,128]
    const_input("b1cat", np.stack([
        np.tile(np.asarray(td_b1, np.float32)[None, :], (128, 1)),
        np.tile(np.asarray(bu_b1, np.float32)[None, :], (128, 1))]))     # [2,128,128]
    const_input("iota", np.tile(np.arange(128, dtype=np.float32)[None, :], (128, 1)))
    const_input("ident", np.eye(128, dtype=np.float32))
    const_input("pw1", np.asarray(pw1, np.float32).reshape(2, 128, 256))
    const_input("pw2", np.asarray(pw2, np.float32).reshape(2, 128, 128))
    b2cat = np.concatenate([np.asarray(bu_b2, np.float32),
                            np.asarray(td_b2, np.float32)])
    q1 = b2cat @ np.asarray(pw1, np.float32)  # [256]
    # rank-2 bias rows: m1 += counts (x) q1 + ones (x) pb1
    const_input("q1row", np.stack([q1, np.asarray(pb1, np.float32)]))  # [2, 256]
    const_input("crow", np.stack([np.asarray(counts, np.float32),
                                  np.ones(N_GRAPHS, np.float32)]))  # [2, 512]
    const_input("ones1", np.ones((1, N_GRAPHS), np.float32))
    const_input("pb2row", np.asarray(pb2, np.float32).reshape(1, 128))

    b1_nonzero = {
        "td": bool(np.any(np.asarray(td_b1) != 0)),
        "bu": bool(np.any(np.asarray(bu_b1) != 0)),
    }

    gq = [0]

    def next_q():
        q = gq[0] % 4
        gq[0] += 1
        return q

    with tile.TileContext(nc) as tc:
        with tc.tile_pool(name="dram", bufs=1, space="DRAM") as dram, \
             tc.tile_pool(name="const", bufs=1) as constp, \
             tc.tile_pool(name="persist", bufs=1) as persist:

            # --------- constants to SBUF ---------
            cw1 = constp.tile([128, 2, 2, 128], BF16, name="cw1")
            nc.sync.dma_start(out=cw1[:], in_=P["W1cat"][:].rearrange(
                "b k p f -> p b k f"))
            cw2 = constp.tile([128, 2, 128], F32, name="cw2")
            nc.sync.dma_start(out=cw2[:], in_=P["W2cat"][:].rearrange("b p f -> p b f"))
            cb1 = constp.tile([128, 2, 128], F32, name="cb1")
            nc.sync.dma_start(out=cb1[:], in_=P["b1cat"][:].rearrange("b p f -> p b f"))
            ciota32 = constp.tile([128, 128], F32, name="ciota32")
            nc.sync.dma_start(out=ciota32[:], in_=P["iota"][:])
            ciota = constp.tile([128, 128], BF16, name="ciota")
            nc.vector.tensor_copy(ciota[:], ciota32[:])
            cident = constp.tile([128, 128], F32, name="cident")
            nc.sync.dma_start(out=cident[:], in_=P["ident"][:])
            cidentb = constp.tile([128, 128], BF16, name="cidentb")
            nc.vector.tensor_copy(cidentb[:], cident[:])
            cpw1 = constp.tile([128, 2, 256], F32, name="cpw1")
            nc.sync.dma_start(out=cpw1[:], in_=P["pw1"][:].rearrange("k p j -> p k j"))
            cpw2 = constp.tile([128, 2, 128], F32, name="cpw2")
            nc.sync.dma_start(out=cpw2[:], in_=P["pw2"][:].rearrange("k p f -> p k f"))
            cq1 = constp.tile([2, 256], F32, name="cq1")
            nc.sync.dma_start(out=cq1[:], in_=P["q1row"][:])
            ccrow = constp.tile([2, N_GRAPHS], F32, name="ccrow")
            nc.sync.dma_start(out=ccrow[:], in_=P["crow"][:])
            cones = constp.tile([1, N_GRAPHS], F32, name="cones")
            nc.sync.dma_start(out=cones[:], in_=P["ones1"][:])
            cpb2 = constp.tile([1, 128], F32, name="cpb2")
            nc.sync.dma_start(out=cpb2[:], in_=P["pb2row"][:])
            cdinv = {}
            for br in ("td", "bu"):
                cdinv[br] = constp.tile([128, NTILES], F32, name=f"cdinv{br}")
                nc.sync.dma_start(out=cdinv[br][:], in_=P[f"dinv_{br}"][:])

            # --------- dram intermediates ---------
            agin2 = dram.tile([NPC, 2 * HIDDEN], BF16, name="agin2")
            # Local outputs: the CC engine delivers straight to local DRAM,
            # so no Shared->Local table copy (and its 51MB of HBM traffic /
            # bandwidth contention) is needed before gathers can start.
            hg2sA = dram.tile([HALF_A, 2 * HIDDEN], BF16, name="hg2sA")
            hg2sB = dram.tile([HALF_B, 2 * HIDDEN], BF16, name="hg2sB")
            ar_in = {}
            ar_out = {}
            for br in ("td", "bu"):
                ar_in[br] = dram.tile([128, N_GRAPHS], BF16, name=f"ar_in{br}")
                ar_out[br] = dram.tile([128, N_GRAPHS], BF16, name=f"ar_out{br}",
                                       addr_space="Shared")

            hploc = persist.tile([128, NTILES, 2 * HIDDEN], BF16, name="hploc")

            # =========== phase A: dense h' = dinv * (x @ W1), both branches ===========
            with tc.tile_pool(name="xT", bufs=1) as xp, \
                 tc.tile_pool(name="psA", bufs=2, space="PSUM") as psA:
                xt = xp.tile([128, 2, NPC], BF16, name="xt")
                for q in range(4):
                    nc.sync.dma_start(
                        out=xt[:, :, q * (NPC // 4):(q + 1) * (NPC // 4)],
                        in_=P["xT"][:].rearrange("(k p) n -> p k n", p=128)[
                            :, :, q * (NPC // 4):(q + 1) * (NPC // 4)])
                for t in range(NTILES):
                    for bi, br in enumerate(("td", "bu")):
                        ps = psA.tile([128, 128], F32, space="PSUM", tag="psA")
                        for k in range(2):
                            nc.tensor.matmul(
                                out=ps[:],
                                lhsT=xt[:, k, t * 128:(t + 1) * 128],
                                rhs=cw1[:, bi, k, :],
                                start=(k == 0), stop=(k == 1),
                            )
                        nc.scalar.activation(
                            out=hploc[:, t, bi * HIDDEN:(bi + 1) * HIDDEN],
                            in_=ps[:],
                            func=mybir.ActivationFunctionType.Copy,
                            scale=cdinv[br][:, t:t + 1])
                        nc.sync.dma_start(
                            out=agin2[t * 128:(t + 1) * 128,
                                      bi * HIDDEN:(bi + 1) * HIDDEN],
                            in_=hploc[:, t, bi * HIDDEN:(bi + 1) * HIDDEN])
                # Two AllGathers: half A (each core's first 25 tiles) fires as
                # soon as those agin2 rows are written; half B follows.  The
                # Shared->Local copy of A overlaps the half-B AllGather, and
                # half-A gathers overlap the half-B copy.
                nc.gpsimd.collective_compute(
                    "AllGather", mybir.AluOpType.bypass,
                    replica_groups=[list(range(NCORES))],
                    ins=[agin2[0:SPLIT, :].opt()],
                    outs=[hg2sA[:].opt()],
                )
                nc.gpsimd.collective_compute(
                    "AllGather", mybir.AluOpType.bypass,
                    replica_groups=[list(range(NCORES))],
                    ins=[agin2[SPLIT:NPC, :].opt()],
                    outs=[hg2sB[:].opt()],
                )
                hg2_halves = [hg2sA, hg2sB]

            # =========== phase B/C: per-branch aggregation + conv2/pool ===========
            with tc.tile_pool(name="psG", bufs=2, space="PSUM") as psG, \
                 tc.tile_pool(name="psY", bufs=1, space="PSUM") as psY, \
                 tc.tile_pool(name="psT", bufs=1, space="PSUM") as psT, \
                 tc.tile_pool(name="idxp", bufs=2) as idxp, \
                 tc.tile_pool(name="stag", bufs=12) as stag, \
                 tc.tile_pool(name="selp", bufs=4) as selp, \
                 tc.tile_pool(name="accp", bufs=1) as accp, \
                 tc.tile_pool(name="h1rp", bufs=1) as h1rp, \
                 tc.tile_pool(name="mtp", bufs=3) as mtp, \
                 tc.tile_pool(name="misc", bufs=2) as misc:

                acc = {}
                h1r = {}
                pooledT_sb = {}
                seg = {}
                for br in ("td", "bu"):
                    acc[br] = accp.tile([128, NTILES, 128], BF16, name=f"acc{br}")
                    h1r[br] = h1rp.tile([128, NTILES, 128], BF16, name=f"h1r{br}")
                    Tch = sched[br]
                    so = np.zeros((NTILES + 1, 2), np.int64)
                    np.cumsum(Tch * 128, axis=0, out=so[1:])
                    seg[br] = so

                idx_max = max(nch[(b2_, h2_)] for b2_ in ("td", "bu")
                              for h2_ in (0, 1))
                emit_ar_td = [None]
                psy = None
                # pass order A-halves first (their table copy lands first),
                # so descriptor generation never waits on the half-B copy.
                for br, h in (("td", 0), ("bu", 0), ("td", 1), ("bu", 1)):
                    bi = 0 if br == "td" else 1
                    seg_off = seg[br]
                    if h == 1 and br == "td":
                        psy = [psY.tile([128, 128], F32, space="PSUM",
                                        tag=f"psY{g}", name=f"psytd{g}")
                               for g in range(4)]
                    elif h == 1 and br == "bu":
                        psy = [psY.tile([128, 128], F32, space="PSUM",
                                        tag=f"psY{g}", name=f"psybu{g}")
                               for g in range(4)]
                    if True:
                        n = nch[(br, h)]
                        idx_sb_h = idxp.tile([128, idx_max * 8], I16,
                                             tag="idx", name=f"idx{br}{h}")
                        nc.sync.dma_start(out=idx_sb_h[:, :n * 8],
                                          in_=P[f"idx_{br}_{h}"][:])
                        drel_sb_h = idxp.tile([128, idx_max], BF16,
                                              tag="drel", name=f"drel{br}{h}")
                        nc.sync.dma_start(out=drel_sb_h[:, :n],
                                          in_=P[f"drel_{br}_{h}"][:])
                        idx_sb = {h: idx_sb_h}
                        drel_sb = {h: drel_sb_h}
                        n_chunks = nch[(br, h)]
                        sizes = _instr_sizes(n_chunks)
                        bi_ = 0 if br == "td" else 1
                        table = hg2_halves[h][:, bi_ * HIDDEN:(bi_ + 1) * HIDDEN]

                        # gather instructions
                        stage_tiles = []
                        e0 = 0
                        gi = 0
                        for n in sizes:
                            st = stag.tile([128, GCH * 128], BF16, tag="stag")
                            nc.gpsimd.dma_gather(
                                out_ap=st[:, :n].rearrange(
                                    "p (c e) -> p c e", e=128),
                                in_ap=table,
                                idxs_ap=idx_sb[h][:, e0 // 16:(e0 + n) // 16],
                                num_idxs=n, num_idxs_reg=n, elem_size=128,
                                elem_step=2 * HIDDEN,
                                single_packet=False, queue_num=next_q(),
                            )
                            stage_tiles.append((st, e0 // 128, n // 128))
                            e0 += n
                            gi += 1
                            if gi == 10 and emit_ar_td[0] is not None:
                                # fire the td AllReduce from deep inside the
                                # bu half-B gather stream: pooled-td is ready
                                # by then, so the trigger never stalls gpsimd.
                                emit_ar_td[0]()
                                emit_ar_td[0] = None

                        def chunk_slice(c):
                            for st, c0, cn in stage_tiles:
                                if c0 <= c < c0 + cn:
                                    return st[:, (c - c0) * 128:(c - c0 + 1) * 128]
                            raise AssertionError

                        # batched selection-matrix build
                        sel_tiles = {}
                        for c0 in range(0, n_chunks, SEL_B):
                            b = min(SEL_B, n_chunks - c0)
                            sel = selp.tile([128, SEL_B * 128], BF16, tag="sel")
                            nc.vector.tensor_tensor(
                                out=sel[:, :b * 128].rearrange(
                                    "p (c d) -> p c d", d=128),
                                in0=drel_sb[h][:, c0:c0 + b].unsqueeze(2)
                                    .to_broadcast([128, b, 128]),
                                in1=ciota[:].unsqueeze(1).to_broadcast([128, b, 128]),
                                op=mybir.AluOpType.is_equal,
                            )
                            sel_tiles[c0] = sel

                        def sel_slice(c):
                            c0 = (c // SEL_B) * SEL_B
                            j = c - c0
                            return sel_tiles[c0][:, j * 128:(j + 1) * 128]

                        # per-tile PSUM accumulation + eviction (all adds on
                        # PE via identity matmuls; evictions on ACT -- keeps
                        # DVE off the shared SBUF port pair so SWDGE
                        # descriptor generation isn't blocked)
                        for t in range(NTILES):
                            ca, cb_ = int(seg_off[t, h]) // 128, int(seg_off[t + 1, h]) // 128
                            ps = psG.tile([128, 128], F32, space="PSUM", tag="psG")
                            for c in range(ca, cb_):
                                nc.tensor.matmul(
                                    out=ps[:], lhsT=sel_slice(c),
                                    rhs=chunk_slice(c),
                                    start=(c == ca), stop=False,
                                )
                            if h == 0:
                                # psum += h'local[t]; acc[t] = psum (bf16)
                                nc.tensor.matmul(
                                    out=ps[:], lhsT=cidentb[:],
                                    rhs=hploc[:, t, bi * HIDDEN:(bi + 1) * HIDDEN],
                                    start=(ca == cb_), stop=True)
                                nc.scalar.activation(
                                    out=acc[br][:, t, :], in_=ps[:],
                                    func=mybir.ActivationFunctionType.Copy)
                            else:
                                # psum += acc[t]; h1r[t] = relu(dinv * psum)
                                nc.tensor.matmul(
                                    out=ps[:], lhsT=cidentb[:], rhs=acc[br][:, t, :],
                                    start=(ca == cb_), stop=True)
                                if b1_nonzero[br]:
                                    tmp2 = misc.tile([128, 128], F32, tag="tmp2")
                                    nc.scalar.activation(
                                        out=tmp2[:], in_=ps[:],
                                        func=mybir.ActivationFunctionType.Copy,
                                        scale=cdinv[br][:, t:t + 1])
                                    nc.vector.tensor_add(tmp2[:], tmp2[:], cb1[:, bi, :])
                                    nc.scalar.activation(
                                        out=h1r[br][:, t, :], in_=tmp2[:],
                                        func=mybir.ActivationFunctionType.Relu)
                                else:
                                    nc.scalar.activation(
                                        out=h1r[br][:, t, :], in_=ps[:],
                                        func=mybir.ActivationFunctionType.Relu,
                                        scale=cdinv[br][:, t:t + 1])
                                # conv2+pool partial: Y[g] += MT[t].T slices @ h1r[t]
                                if t % 2 == 0:
                                    tn = min(2, NTILES - t)
                                    mt = mtp.tile([128, 2 * N_GRAPHS], BF16, tag="mt")
                                    nc.sync.dma_start(
                                        out=mt[:, :tn * N_GRAPHS],
                                        in_=P[f"MT_{br}"][
                                            :, t * N_GRAPHS:(t + tn) * N_GRAPHS])
                                mtoff = (t % 2) * N_GRAPHS
                                for g in range(4):
                                    nc.tensor.matmul(
                                        out=psy[g][:],
                                        lhsT=mt[:, mtoff + g * 128:mtoff + (g + 1) * 128],
                                        rhs=h1r[br][:, t, :],
                                        start=(t == 0), stop=(t == NTILES - 1),
                                        skip_group_check=True,
                                    )

                    if h != 1:
                        continue
                    # transpose Y -> YT [128f, 512g]
                    yt = misc.tile([128, N_GRAPHS], F32, tag="yt")
                    for g in range(4):
                        ysb = misc.tile([128, 128], F32, tag="ysb")
                        nc.scalar.activation(out=ysb[:], in_=psy[g][:],
                                             func=mybir.ActivationFunctionType.Copy)
                        pst = psT.tile([128, 128], F32, space="PSUM", tag="psT")
                        nc.tensor.transpose(out=pst[:], in_=ysb[:], identity=cident[:])
                        nc.scalar.activation(out=yt[:, g * 128:(g + 1) * 128],
                                             in_=pst[:],
                                             func=mybir.ActivationFunctionType.Copy)
                    # pooledT = W2^T-contraction: [128fo, 512g]
                    psp = psT.tile([128, N_GRAPHS], F32, space="PSUM", tag="psp")
                    nc.tensor.matmul(out=psp[:], lhsT=cw2[:, bi, :], rhs=yt[:],
                                     start=True, stop=True)
                    pooledT_sb[br] = misc.tile([128, N_GRAPHS], BF16, tag=f"pool{br}", name=f"pool{br}")
                    nc.scalar.activation(out=pooledT_sb[br][:], in_=psp[:],
                                         func=mybir.ActivationFunctionType.Copy)
                    nc.sync.dma_start(out=ar_in[br][:], in_=pooledT_sb[br][:])

                    def _mk_ar(br=br):
                        def _emit():
                            nc.gpsimd.collective_compute(
                                "AllReduce", mybir.AluOpType.add,
                                replica_groups=[list(range(NCORES))],
                                ins=[ar_in[br][:].opt()],
                                outs=[ar_out[br][:].opt()],
                            )
                        return _emit
                    if br == "td":
                        emit_ar_td[0] = _mk_ar()
                    else:
                        _mk_ar()()

            # =========== phase D: MLP head (replicated) ===========
            with tc.tile_pool(name="psM", bufs=1, space="PSUM") as psM, \
                 tc.tile_pool(name="mlp", bufs=1) as mlp:
                catb = mlp.tile([128, 2, N_GRAPHS], BF16, name="catb")
                # cat order is [bu, td] -> slot 0 = bu, slot 1 = td
                nc.sync.dma_start(out=catb[:, 0, :], in_=ar_out["bu"][:])
                nc.sync.dma_start(out=catb[:, 1, :], in_=ar_out["td"][:])
                cat = mlp.tile([128, 2, N_GRAPHS], F32, name="cat")
                nc.vector.tensor_copy(cat[:], catb[:])
                m1 = []
                for j in range(2):
                    pm = psM.tile([128, N_GRAPHS], F32, space="PSUM", tag=f"psM{j}", name=f"pm{j}")
                    for k in range(2):
                        nc.tensor.matmul(
                            out=pm[:], lhsT=cpw1[:, k, j * 128:(j + 1) * 128],
                            rhs=cat[:, k, :], start=(k == 0), stop=False,
                            skip_group_check=True)
                    # rank-2 bias: [q1; pb1-via-q1? q1 already includes pb1] x [counts; ones]
                    nc.tensor.matmul(
                        out=pm[:], lhsT=cq1[:2, j * 128:(j + 1) * 128],
                        rhs=ccrow[:2, :], start=False, stop=True,
                        skip_group_check=True)
                    m1t = mlp.tile([128, N_GRAPHS], F32, name=f"m1t{j}")
                    nc.scalar.activation(out=m1t[:], in_=pm[:],
                                         func=mybir.ActivationFunctionType.Relu)
                    m1.append(m1t)
                pm2 = psM.tile([128, N_GRAPHS], F32, space="PSUM", tag="psM2")
                for j in range(2):
                    nc.tensor.matmul(out=pm2[:], lhsT=cpw2[:, j, :], rhs=m1[j][:],
                                     start=(j == 0), stop=False,
                                     skip_group_check=True)
                nc.tensor.matmul(out=pm2[:], lhsT=cpb2[:1, :], rhs=cones[:1, :],
                                 start=False, stop=True, skip_group_check=True)
                o_sb = mlp.tile([128, N_GRAPHS], F32, name="o_sb")
                nc.vector.tensor_copy(o_sb[:], pm2[:])
                nc.sync.dma_start(out=out_ext[:], in_=o_sb[:])

    return consts_np


# ---------------------------------------------------------------- entrypoint
def kernel(x, edge_index, batch, num_graphs,
           td_W1, td_b1, td_W2, td_b2,
           bu_W1, bu_b1, bu_W2, bu_b2,
           pw1, pb1, pw2, pb2):
    _patch_tile_drain()
    x = np.asarray(x)
    edge_index = np.asarray(edge_index)
    batch = np.asarray(batch)

    counts = np.bincount(np.asarray(batch, np.int64),
                         minlength=N_GRAPHS).astype(np.float32)
    sched, in_maps, counts = _prep(x, edge_index, batch, td_W1, bu_W1,
                                   td_b2, bu_b2, pw1, pb1)

    nc = bacc.Bacc("TRN2", num_devices=NCORES, num_swdge_queues=4)
    weights = (td_W1, td_b1, td_W2, td_b2, bu_W1, bu_b1, bu_W2, bu_b2,
               pw1, pb1, pw2, pb2, counts)
    consts_np = _build(nc, sched, weights)
    nc.finalize()

    for m in in_maps:
        m.update(consts_np)

    core_ids = list(range(NCORES))
    kw = {}
    td = os.environ.get("BIGCN_TMPDIR")
    if td:
        os.makedirs(td, exist_ok=True)
        kw["tmpdir"] = td
    res = run_bass_kernel_spmd(nc, in_maps, core_ids, trace=_TRACE, **kw)
    if _TRACE and res.exec_time_ns is not None:
        print(f"HW exec time: {res.exec_time_ns} ns")

    outT = res.results[0]["out"]          # [128 feat, 512 graphs]
    return np.ascontiguousarray(outT.T).astype(np.float32)

